# revision 8
# baseline (speedup 1.0000x reference)
"""Multi-head self-attention (B=2, S=2048, D=768, H=12) on 8 trn2 NeuronCores.

Sharding: core c = 4*b + g handles batch b and head-group g (3 heads = 192 of
the 768 model dims). Weights are column-split (wq/wk/wv) and row-split (wo);
each core emits a partial (2048, 768) output; the host sums the 4 group
partials per batch and adds bo.

Device-side dataflow is transpose-free: inputs arrive pre-transposed (D, S),
so projections produce Q^T/K^T in (head_dim, S) layout which feeds the
logits matmul directly; softmax is computed as exp(logits/8) without
max-subtraction (logits are ~N(0,1), exp cannot overflow) with denominators
obtained from a ones-column appended to V in the context matmul; the context
comes out transposed (dims, S), which is exactly the stationary operand the
output projection needs.

Matmul operands are bfloat16 (1 cyc/row streaming, half the DMA/SBUF bytes
of fp32); accumulation stays fp32 in PSUM. Inputs/weights are converted to
bf16 on the host. fp8 was measured and rejected: e4m3 anywhere in the
datapath (even V only) pushes rel-err past the 2e-2 gate.

Ramp/tail scheduling (from NTFF trace analysis):
- DMA issue order is latency-driven: kx/qx chunk-0 and wk/wq interleave as
  the first transfers on the two hardware rings (sync + scalar) so the Q
  path does not queue behind the whole K/V stream; the DMA-completion
  semaphore pool is ~10 deep, so the first ten issues must be exactly the
  ten transfers slot (0,0) needs. Biases ride the scalar ring (the gpsimd
  software queue adds ~1.5us each); wo alone stays on gpsimd.
- PE warm-up dummies bridge the preamble-to-data window so the activity
  monitor never sees an idle PE and never halves the clock (a k=4 HAM
  window costs ~5us of doubled matmul time).
- Output path: Y tiles evict via the idle Pool engine (DVE copies were
  pacing the output projection at ~1.1us per 128-row block) and store as
  fp16 (halves the final-store tail; partial-sum fp16 quantization is
  ~6e-5, negligible against the 2e-2 gate).
- The last (chunk, head) slot is processed in two 256-query halves so the
  first half's output projection overlaps the second half's attention,
  shortening the serial tail to one half-chunk.
"""
import numpy as np
from contextlib import ExitStack

import ml_dtypes

import concourse.bacc as bacc
import concourse.mybir as mybir
import concourse.tile as tile
from concourse import bass_utils

# Problem shape (hardcoded per contract).
B, S, D, H, DH = 2, 2048, 768, 12, 64
NCORES = 8
NG = 4            # head groups
HG = H // NG      # heads per group (3)
G = HG * DH       # model dims per group (192)
SC = 512          # query-chunk length
NQ = S // SC      # 4 chunks
KB = 128          # key-block length
NKB = S // KB     # 16 blocks
KT6 = D // 128    # 6 contraction tiles for the projections
SEG = 2 * DH      # V segment width per head: 64 V columns + 64 ones columns
HC = 256          # query half-chunk (tail split of the last slot)
FP32 = mybir.dt.float32
F16 = mybir.dt.float16
CDT = mybir.dt.bfloat16   # matmul-operand dtype
NP_CDT = ml_dtypes.bfloat16

AF = mybir.ActivationFunctionType
ALU = mybir.AluOpType

_CACHE: dict = {}


def _build():
    nc = bacc.Bacc("TRN2", target_bir_lowering=False, debug=False)

    qT = nc.dram_tensor("qT", [NQ, 128, KT6, SC], CDT, kind="ExternalInput")
    kT = nc.dram_tensor("kT", [NQ, 128, KT6, SC], CDT, kind="ExternalInput")
    vT = nc.dram_tensor("vT", [NKB, 128, KT6, KB], CDT, kind="ExternalInput")
    wq = nc.dram_tensor("wq", [128, KT6, G], CDT, kind="ExternalInput")
    wk = nc.dram_tensor("wk", [128, KT6, G], CDT, kind="ExternalInput")
    wv = nc.dram_tensor("wv", [128, KT6, G], CDT, kind="ExternalInput")
    wo = nc.dram_tensor("wo", [G, D], CDT, kind="ExternalInput")
    bq = nc.dram_tensor("bq", [G, 1], FP32, kind="ExternalInput")
    bk = nc.dram_tensor("bk", [G, 1], FP32, kind="ExternalInput")
    yp = nc.dram_tensor("yp", [S, D], F16, kind="ExternalOutput")

    with tile.TileContext(nc) as tc, ExitStack() as ctx:
        const = ctx.enter_context(tc.tile_pool(name="const", bufs=1))
        xin = ctx.enter_context(tc.tile_pool(name="xin", bufs=4))
        qxp = ctx.enter_context(tc.tile_pool(name="qxp", bufs=2))
        qtp = ctx.enter_context(tc.tile_pool(name="qtp", bufs=2))
        ppool = ctx.enter_context(tc.tile_pool(name="ppool", bufs=2))
        ctxp = ctx.enter_context(tc.tile_pool(name="ctxp", bufs=2))
        ypool = ctx.enter_context(tc.tile_pool(name="ypool", bufs=2))
        den = ctx.enter_context(tc.tile_pool(name="den", bufs=2))
        ps_proj = ctx.enter_context(tc.tile_pool(name="ps_proj", bufs=2, space="PSUM"))
        ps_log = ctx.enter_context(tc.tile_pool(name="ps_log", bufs=2, space="PSUM"))
        ps_ctx = ctx.enter_context(tc.tile_pool(name="ps_ctx", bufs=2, space="PSUM"))

        # ---- weights / constants ------------------------------------------
        # The first transfers on the two hardware DMA rings are exactly the
        # slot-(0,0) needs, interleaved K-path/Q-path so neither projection
        # chain queues behind the other: sync ring carries kx c0 then qx c0
        # (two 3-t halves each, matching the 3-t matmul units); the scalar
        # ring interleaves wk/wq 3-t halves, then the (tiny) biases, then
        # wv. wo is not needed until the first output projection (~45us)
        # and rides the slow-but-idle gpsimd software queue.
        wk_sb = const.tile([128, KT6, G], CDT)
        wq_sb = const.tile([128, KT6, G], CDT)
        for j in range(2):
            nc.scalar.dma_start(
                wk_sb[:, 3 * j:3 * j + 3, :], wk.ap()[:, 3 * j:3 * j + 3, :]
            )
            nc.scalar.dma_start(
                wq_sb[:, 3 * j:3 * j + 3, :], wq.ap()[:, 3 * j:3 * j + 3, :]
            )
        bk0 = const.tile([128, 1], FP32)
        nc.scalar.dma_start(bk0[:], bk.ap()[0:128, :])
        bk1 = const.tile([64, 1], FP32)
        nc.scalar.dma_start(bk1[:], bk.ap()[128:G, :])
        bq0 = const.tile([128, 1], FP32)
        nc.scalar.dma_start(bq0[:], bq.ap()[0:128, :])
        bq1 = const.tile([64, 1], FP32)
        nc.scalar.dma_start(bq1[:], bq.ap()[128:G, :])
        wv_sb = const.tile([128, KT6, G], CDT)
        nc.scalar.dma_start(wv_sb[:], wv.ap()[:, :, :])
        wo_sb0 = const.tile([128, D], CDT)
        nc.gpsimd.dma_start(wo_sb0[:], wo.ap()[0:128, :])
        wo_sb1 = const.tile([128, D], CDT)
        nc.gpsimd.dma_start(wo_sb1[0:64, :], wo.ap()[128:G, :])
        # bf16 consts via fp32 memset + CAST-copy (uniform with f32r-era code).
        ones_f32 = const.tile([128, NKB], FP32)
        nc.vector.memset(ones_f32[:], 1.0)
        zero_f32 = const.tile([128, 1], FP32)
        nc.vector.memset(zero_f32[:], 0.0)

        def zero_fill(dst_ap, parts, cols):
            nc.vector.tensor_copy(
                dst_ap, zero_f32[0:parts, 0:1].to_broadcast((parts, cols))
            )

        # PE warm-up: the clock gate releases only after a sustained-busy
        # window, and the initial weight/input DMAs would otherwise leave
        # the PE idle; idling also makes the activity monitor clamp the
        # clock to half rate right as the first real matmuls issue. Burn
        # dependency-free matmuls on zeros sized to bridge until the first
        # kx/wk slices land (~2us after the preamble).
        wsrc = const.tile([128, SC], CDT)
        zero_fill(wsrc[:, :], 128, SC)
        wps = ps_log.tile([128, 2, SC], FP32, tag="pl", name="warmps")
        for _ in range(6):
            nc.tensor.matmul(
                wps[:, 0, :], wsrc[:, 0:128], wsrc[:, :],
                start=True, stop=True,
            )

        # K^T per head, zero-padded to a full 128-partition contraction.
        # Partition placement matches the stacked Q^T tiles, so the padding
        # rows multiply zeros (or real rows multiply zero Q halves) and
        # every logits matmul runs with a full-height stationary — a
        # half-height (K=64) stationary makes the PE look half-idle to the
        # activity monitor, which then clamps the clock to half rate.
        # Only KTz0's pad is zeroed now; the other fills are deferred past
        # the first projections so the DVE queue can evict psum promptly.
        KTz0 = const.tile([128, S], CDT)   # [K_h0^T ; 0]
        KTz1 = const.tile([128, S], CDT)   # [0 ; K_h1^T]
        KTz2 = const.tile([128, S], CDT)   # [K_h2^T ; 0]
        zero_fill(KTz0[64:128, :], 64, S)
        # V blocks + 64 ones columns per head: the context matmul then
        # replicates the softmax denominator across 64 psum partitions
        # (stationary width is free), so normalization needs no
        # partition-broadcast DMA chain at all.
        Vg = const.tile([128, NKB, HG * SEG], CDT)

        def mk_fill_units():
            # Late const fills, packaged as filler units so they interleave
            # with slot (0,0)'s eviction traffic on the DVE queue. KTz2 is
            # first read in slot (0,2), Vg's ones in slot (0,1)'s context
            # matmuls, wo_sb1 at the first output projection.
            units = [
                lambda: zero_fill(KTz2[64:128, :], 64, S),
                lambda: zero_fill(wo_sb1[64:128, :], 64, D),
            ]
            for h in range(HG):
                units.append(lambda h=h: nc.vector.tensor_copy(
                    Vg[:, :, h * SEG + DH:(h + 1) * SEG],
                    ones_f32[:, 0:1, None].to_broadcast((128, NKB, DH)),
                ))
            return units

        mblocks = ((128, 0), (64, 128))  # (rows, row-offset) of the 192 dims

        # ---- K^T / V projections as emission units -----------------------
        # These are DMA-bound; instead of running them as serial phases
        # (PE half-idle, HAM re-throttles), they are spread as PE filler
        # into the first stream slots, hiding the loads under attention.
        def kt_units(c):
            kx = xin.tile([128, KT6, SC], CDT, tag="kx", name=f"kx_{c}")
            for j in range(2):
                nc.sync.dma_start(
                    kx[:, 3 * j:3 * j + 3, :], kT.ap()[c, :, 3 * j:3 * j + 3, :]
                )
            state = {}
            units = []

            def mk_mm(m, mp, mo, tpl):
                def emit():
                    if m not in state:
                        state[m] = ps_proj.tile(
                            [128, SC], FP32, tag="pp", name=f"ktps_{c}_{m}"
                        )
                    ps = state[m]
                    for t in tpl:
                        nc.tensor.matmul(
                            ps[:mp, :], wk_sb[:, t, mo:mo + mp], kx[:, t, :],
                            start=(t == 0), stop=(t == KT6 - 1),
                        )
                return emit

            def mk_evict(m, mp):
                def emit():
                    sl = slice(c * SC, (c + 1) * SC)
                    if m == 0:
                        nc.vector.tensor_scalar_add(
                            KTz0[0:64, sl], state[m][0:64, :], bk0[0:64, :]
                        )
                        nc.vector.tensor_scalar_add(
                            KTz1[64:128, sl], state[m][64:128, :],
                            bk0[64:128, :],
                        )
                    else:
                        nc.vector.tensor_scalar_add(
                            KTz2[0:64, sl], state[m][0:64, :], bk1[0:64, :]
                        )
                return emit

            for m, (mp, mo) in enumerate(mblocks):
                for tpl in ((0, 1, 2), (3, 4, 5)):
                    units.append(mk_mm(m, mp, mo, tpl))
                units.append(mk_evict(m, mp))
            return units

        def v_units(sb):
            vx = xin.tile([128, KT6, KB], CDT, tag="vx", name=f"vx_{sb}")
            nc.sync.dma_start(vx[:], vT.ap()[sb])
            state = {}
            units = []

            def mk_mm(tpl, last):
                def emit():
                    if "ps" not in state:
                        state["ps"] = ps_proj.tile(
                            [128, G], FP32, tag="pp", name=f"vps_{sb}"
                        )
                    ps = state["ps"]
                    for t in tpl:
                        nc.tensor.matmul(
                            ps[:], vx[:, t, :], wv_sb[:, t, :],
                            start=(t == 0), stop=(last and t == KT6 - 1),
                        )
                return emit

            def mk_evict():
                def emit():
                    # one strided copy lands all three heads' V columns
                    nc.vector.tensor_copy(
                        Vg[:, sb, :].rearrange(
                            "p (h s) -> p h s", h=HG
                        )[:, :, 0:DH],
                        state["ps"][:, :].rearrange(
                            "p (h s) -> p h s", h=HG
                        ),
                    )
                return emit

            units.append(mk_mm((0, 1, 2), False))
            units.append(mk_mm((3, 4, 5), True))
            units.append(mk_evict())
            return units

        # ---- phase 3: software-pipelined head stream ---------------------
        # Heads form one flat stream across chunks. Each slot interleaves
        # head i's logits+exp with head i-1's context matmuls so PE and ACT
        # both stay fed (in-order engines execute in emission order). The
        # normalization chain of head i-1 is emitted at slot end; the output
        # projection of a finished chunk is emitted one slot later, after
        # its normalization latency has been hidden under a full slot.
        QT = {}     # qc -> (QT0, QT1)
        CT = {}     # qc -> (ctxT0, ctxT1)

        KTZ = (KTz0, KTz1, KTz2)

        def head_slices(qc, h):
            qt0, qt1 = QT[qc]
            return KTZ[h], qt0 if h < 2 else qt1

        def qt_units(qc):
            # QT projection broken into emission units (PE filler). The qx
            # DMA and tile allocations happen now; matmuls are emitted as
            # the units are drained inside a kb2 loop.
            qx = qxp.tile([128, KT6, SC], CDT, tag="qx", name=f"qx_{qc}")
            for j in range(2):
                nc.sync.dma_start(
                    qx[:, 3 * j:3 * j + 3, :], qT.ap()[qc, :, 3 * j:3 * j + 3, :]
                )
            QT0 = qtp.tile([128, SC], CDT, tag="qt0", name=f"QT0_{qc}")
            QT1 = qtp.tile([128, SC], CDT, tag="qt1", name=f"QT1_{qc}")
            if qc < 2:
                # The qt1 pool has two buffers; their pad rows are zeroed on
                # first use and never overwritten (the m1 evict writes rows
                # 0:64 only), so chunks 2/3 reuse already-zeroed pads.
                zero_fill(QT1[64:128, :], 64, SC)
            QT[qc] = (QT0, QT1)
            units = []
            state = {}

            def mk_mm(m, mp, mo, tpl):
                def emit():
                    if m not in state:
                        state[m] = ps_proj.tile(
                            [128, SC], FP32, tag="pp", name=f"qtps_{qc}_{m}"
                        )
                    ps = state[m]
                    for t in tpl:
                        nc.tensor.matmul(
                            ps[:mp, :], wq_sb[:, t, mo:mo + mp], qx[:, t, :],
                            start=(t == 0), stop=(t == KT6 - 1),
                        )
                return emit

            def mk_evict(m, mp):
                def emit():
                    dst = QT0 if m == 0 else QT1
                    bias = bq0 if m == 0 else bq1
                    nc.vector.tensor_scalar_add(
                        dst[0:mp, :], state[m][0:mp, :], bias[0:mp, :]
                    )
                return emit

            for m, (mp, mo) in enumerate(mblocks):
                for tpl in ((0, 1, 2), (3, 4, 5)):
                    units.append(mk_mm(m, mp, mo, tpl))
                units.append(mk_evict(m, mp))
            return units

        def emit_norm(qc, h, pc, cs=slice(0, SC)):
            # Normalization: the denominator already sits replicated on psum
            # partitions DH..SEG (ones columns in Vg), so this is just a
            # wide approximate reciprocal plus a column-wise scale — three
            # DVE ops, no DMA hops. (The custom-DVE reciprocal must read
            # SBUF, not PSUM, hence the staging copy.)
            n = cs.stop - cs.start
            dsb = den.tile([64, SC], FP32, tag="dsb")
            nc.vector.tensor_copy(dsb[:, 0:n], pc[DH:SEG, cs])
            rbc = den.tile([64, SC], FP32, tag="rbc")
            nc.vector.reciprocal_approx_fast(rbc[:, 0:n], dsb[:, 0:n])
            ctxT0, ctxT1 = CT[qc]
            cdst = ctxT0[64 * h:64 * h + 64, cs] if h < 2 else ctxT1[0:64, cs]
            nc.vector.tensor_tensor(cdst, pc[0:DH, cs], rbc[:, 0:n], ALU.mult)

        def y_units(qc, alt_from=None):
            # Output projection as emission units (PE filler): 8 units of
            # two accumulating matmuls + psum eviction; each half-chunk's
            # DMA is split per 128-row block so the final store drains in
            # small pieces. The store is fp16 — half the bytes of the old
            # fp32 partials. Units >= alt_from alternate their psum
            # eviction between DVE and ACT (the tail units run after the
            # last exp, when ACT is idle, and DVE copies would otherwise
            # pace the serial drain at ~560ns per block).
            ctxT0, ctxT1 = CT[qc]
            ytiles = {}
            units = []

            def mk_unit(half, m, nh, on_act):
                def emit():
                    if half not in ytiles:
                        ytiles[half] = ypool.tile(
                            [128, 2, D], F16, tag="Y", name=f"Yt_{qc}_{half}"
                        )
                    Yt = ytiles[half]
                    sb = half * 2 + m
                    py = ps_proj.tile(
                        [128, D // 2], FP32, tag="pp", name=f"yps_{qc}_{sb}_{nh}"
                    )
                    nc.tensor.matmul(
                        py[:],
                        ctxT0[:, sb * 128:(sb + 1) * 128],
                        wo_sb0[:, nh * (D // 2):(nh + 1) * (D // 2)],
                        start=True, stop=False,
                    )
                    nc.tensor.matmul(
                        py[:],
                        ctxT1[:, sb * 128:(sb + 1) * 128],
                        wo_sb1[:, nh * (D // 2):(nh + 1) * (D // 2)],
                        start=False, stop=True,
                    )
                    ydst = Yt[:, m, nh * (D // 2):(nh + 1) * (D // 2)]
                    if on_act:
                        nc.scalar.activation(ydst, py[:], AF.Copy)
                    else:
                        nc.vector.tensor_copy(ydst, py[:])
                    if nh == 1:
                        r0 = qc * SC + half * 256 + m * 128
                        nc.sync.dma_start(
                            yp.ap()[r0:r0 + 128, :], Yt[:, m, :]
                        )
                return emit

            i = 0
            for half in range(2):
                for m in range(2):
                    for nh in range(2):
                        on_act = alt_from is not None and i >= alt_from and i % 2 == 1
                        units.append(mk_unit(half, m, nh, on_act))
                        i += 1
            return units

        def emit_y(qc):
            for u in y_units(qc):
                u()

        def emit_ctx_pair(prev, kb2, cs=slice(0, SC)):
            qc_p, h_p, P_p, pc_p = prev
            for j in range(2):
                kb = 2 * kb2 + j
                nc.tensor.matmul(
                    pc_p[0:SEG, cs],
                    Vg[:, kb, h_p * SEG:(h_p + 1) * SEG],
                    P_p[:, kb, cs],
                    start=(kb == 0), stop=(kb == NKB - 1),
                )

        stream = [(qc, h) for qc in range(NQ) for h in range(HG)]
        prev = None      # (qc, h, P, pc) of the head whose ctx is in flight

        # Slot (0,0) consumes only the m0 halves (heads 0/1 K, head-0/1 Q),
        # so just kt0.m0 + qt0.m0 run as the pre-stream block; every other
        # projection half streams in as filler placed one slot before its
        # first consumer. QT m0 feeds slot (qc,0) and QT m1 only (qc,2),
        # letting the QT prefetch straddle two otherwise-empty slots.
        KT_U = {0: kt_units(0)}
        QT_U = {0: qt_units(0)}
        for u in KT_U[0][0:3]:
            u()
        for u in QT_U[0][0:3]:
            u()
        zero_fill(KTz1[0:64, :], 64, S)
        KT_U.update({c: kt_units(c) for c in (1, 2, 3)})

        NIT = NKB // 2
        for si, (qc, h) in enumerate(stream):
            last_slot = si == len(stream) - 1
            if h == 0:
                ctxT0_n = ctxp.tile([128, SC], CDT, tag="c0",
                                    name=f"ctxT0_{qc}")
                ctxT1_n = ctxp.tile([128, SC], CDT, tag="c1",
                                    name=f"ctxT1_{qc}")
                if qc < 2:
                    # two-buffer pool: pads zeroed on first use only (the
                    # norm writes rows 0:64; wo_sb1's zero pad rows guard
                    # the contraction anyway — this is belt-and-braces for
                    # Inf/NaN junk on a cold SBUF).
                    zero_fill(ctxT1_n[64:128, :], 64, SC)
                CT[qc] = (ctxT0_n, ctxT1_n)
            filler = []
            start_iter = 0
            if (qc, h) == (0, 0):
                filler.extend(KT_U[1][0:3])
                filler.extend(v_units(0) + v_units(1))
                filler.extend(KT_U[2][0:3])
                filler.extend(v_units(2) + v_units(3))
                filler.extend(KT_U[3][0:3])
                filler.extend(v_units(4) + v_units(5))
                filler.extend(mk_fill_units())
            elif (qc, h) == (0, 1):
                filler.extend(KT_U[0][3:6])
                filler.extend(QT_U[0][3:6])
                for sb in range(6, NKB):
                    filler.extend(v_units(sb))
            elif (qc, h) == (0, 2):
                for c in (1, 2, 3):
                    filler.extend(KT_U[c][3:6])
                QT_U[1] = qt_units(1)
                filler.extend(QT_U[1][0:3])
            elif h == 2 and qc + 1 < NQ:
                QT_U[qc + 1] = qt_units(qc + 1)
                filler.extend(QT_U[qc + 1][0:3])
            elif h == 0 and qc >= 1:
                filler = list(QT_U[qc][3:6])
            elif h == 1 and qc >= 1:
                filler = y_units(qc - 1)
                start_iter = 2

            kt_t, qt_t = head_slices(qc, h)

            if last_slot:
                # The last slot has no successor to hide its normalization
                # and output projection under, so it is processed in two
                # 256-query halves: half A's y-projection units become PE
                # filler for half B's attention, and only half B's
                # normalization + 8 y-units + 2 fp16 stores drain serially.
                P = ppool.tile([128, NKB, SC], CDT, tag="P")
                # y_units yields 8 units: 4 per 256-query half. Half A's
                # four units become PE filler during half B's attention
                # (their ctx columns 0:256 are normalized by then); half
                # B's four drain after its own normalization.
                yu = y_units(qc, alt_from=4)
                for half in range(2):
                    cs = slice(half * HC, (half + 1) * HC)
                    pc_h = ps_ctx.tile([128, SC], FP32, tag="pc")
                    hfill = yu[0:4] if half == 1 else []
                    for kb2 in range(NIT):
                        pl = ps_log.tile([128, 2, SC], FP32, tag="pl")
                        for j in range(2):
                            kb = 2 * kb2 + j
                            nc.tensor.matmul(
                                pl[:, j, cs],
                                kt_t[:, kb * KB:(kb + 1) * KB],
                                qt_t[:, cs],
                                start=True, stop=True,
                            )
                        nc.scalar.activation(
                            P[:, 2 * kb2:2 * kb2 + 2, cs], pl[:, :, cs],
                            AF.Exp, scale=1.0 / np.sqrt(DH)
                        )
                        if half == 0 and prev is not None:
                            emit_ctx_pair(prev, kb2)
                        if hfill:
                            n = -(-len(hfill) // (NIT - kb2))
                            for _ in range(n):
                                hfill.pop(0)()
                        if kb2 >= 1:
                            emit_ctx_pair((qc, h, P, pc_h), kb2 - 1, cs)
                    emit_ctx_pair((qc, h, P, pc_h), NIT - 1, cs)
                    if half == 0 and prev is not None:
                        emit_norm(prev[0], prev[1], prev[3])
                    emit_norm(qc, h, pc_h, cs)
                # keep the PE busy through the final normalization latency
                # so the activity monitor does not clamp the clock.
                dps = ps_log.tile([128, 2, SC], FP32, tag="pl", name="drainps")
                for _ in range(6):
                    nc.tensor.matmul(
                        dps[:, 0, :], wsrc[:, 0:128], wsrc[:, :],
                        start=True, stop=True,
                    )
                for u in yu[4:8]:
                    u()
                break

            P = ppool.tile([128, NKB, SC], CDT, tag="P")
            for kb2 in range(NIT):
                pl = ps_log.tile([128, 2, SC], FP32, tag="pl")
                for j in range(2):
                    kb = 2 * kb2 + j
                    nc.tensor.matmul(
                        pl[:, j, :],
                        kt_t[:, kb * KB:(kb + 1) * KB],
                        qt_t[:, :],
                        start=True, stop=True,
                    )
                nc.scalar.activation(
                    P[:, 2 * kb2:2 * kb2 + 2, :], pl[:],
                    AF.Exp, scale=1.0 / np.sqrt(DH)
                )
                if filler and kb2 >= start_iter:
                    n = -(-len(filler) // (NIT - kb2))
                    for _ in range(n):
                        filler.pop(0)()
                if prev is not None:
                    emit_ctx_pair(prev, kb2)
            if prev is not None:
                emit_norm(prev[0], prev[1], prev[3])
            for u in filler:
                u()
            pc_cur = ps_ctx.tile([128, SC], FP32, tag="pc")
            prev = (qc, h, P, pc_cur)

    nc.compile()
    return nc


def _get_nc():
    if "nc" not in _CACHE:
        _CACHE["nc"] = _build()
    return _CACHE["nc"]


def _tile_x(xb, chunk):
    # x (S, D) -> x^T tiled (S/chunk, 128, KT6, chunk), contiguous, bf16
    xt = np.asarray(xb, dtype=np.float32).T.astype(NP_CDT)
    return np.ascontiguousarray(
        xt.reshape(KT6, 128, S // chunk, chunk).transpose(2, 1, 0, 3)
    )


def _tile_w(w):
    # (D, G) -> (128, KT6, G) contiguous, bf16
    w = np.asarray(w, dtype=np.float32).astype(NP_CDT)
    return np.ascontiguousarray(w.reshape(KT6, 128, G).transpose(1, 0, 2))


def _in_maps(v, k, q, wq, bq, wk, bk, wv, bv, wo, bo):
    f32 = lambda a: np.ascontiguousarray(np.asarray(a, dtype=np.float32))
    bf16 = lambda a: np.ascontiguousarray(
        np.asarray(a, dtype=np.float32).astype(NP_CDT)
    )
    qTb = [_tile_x(q[b], SC) for b in range(B)]
    kTb = [_tile_x(k[b], SC) for b in range(B)]
    vTb = [_tile_x(v[b], KB) for b in range(B)]
    maps = []
    for c in range(NCORES):
        b, g = divmod(c, NG)
        cols = slice(g * G, (g + 1) * G)
        maps.append({
            "qT": qTb[b],
            "kT": kTb[b],
            "vT": vTb[b],
            "wq": _tile_w(np.asarray(wq)[:, cols]),
            "wk": _tile_w(np.asarray(wk)[:, cols]),
            "wv": _tile_w(np.asarray(wv)[:, cols]),
            "wo": bf16(wo[cols, :]),
            "bq": f32(np.asarray(bq)[cols].reshape(G, 1)),
            "bk": f32(np.asarray(bk)[cols].reshape(G, 1)),
        })
    return maps


def kernel(v, k, q, wq, bq, wk, bk, wv, bv, wo, bo, _trace=False):
    nc = _get_nc()
    in_maps = _in_maps(v, k, q, wq, bq, wk, bk, wv, bv, wo, bo)
    res = bass_utils.run_bass_kernel_spmd(
        nc, in_maps, core_ids=list(range(NCORES)), trace=_trace
    )
    # softmax weights sum to 1, so the V bias shifts ctx by exactly bv;
    # its contribution to the output is the constant row bv @ wo + bo.
    corr = (np.asarray(bv, dtype=np.float64) @ np.asarray(wo, dtype=np.float64)
            + np.asarray(bo, dtype=np.float64)).astype(np.float32)
    out = np.empty((B, S, D), dtype=np.float32)
    for b in range(B):
        acc = res.results[4 * b]["yp"].astype(np.float32)
        for g in range(1, NG):
            acc = acc + res.results[4 * b + g]["yp"].astype(np.float32)
        out[b] = acc + corr[None, :]
    if _trace:
        kernel.last_result = res
    return out


# revision 17
# speedup vs baseline: 1.0170x; 1.0170x over previous
"""Multi-head self-attention (B=2, S=2048, D=768, H=12) on 8 trn2 NeuronCores.

Sharding: core c = 4*b + g handles batch b and head-group g (3 heads = 192 of
the 768 model dims). Weights are column-split (wq/wk/wv) and row-split (wo);
each core emits a partial (2048, 768) output; the host sums the 4 group
partials per batch and adds bo.

Device-side dataflow is transpose-free: inputs arrive pre-transposed (D, S),
so projections produce Q^T/K^T in (head_dim, S) layout which feeds the
logits matmul directly; softmax is computed as exp(logits/8) without
max-subtraction (logits are ~N(0,1), exp cannot overflow) with denominators
obtained from a ones-column appended to V in the context matmul; the context
comes out transposed (dims, S), which is exactly the stationary operand the
output projection needs.

Matmul operands are bfloat16 (1 cyc/row streaming, half the DMA/SBUF bytes
of fp32); accumulation stays fp32 in PSUM. Inputs/weights are converted to
bf16 on the host. fp8 was measured and rejected: e4m3 anywhere in the
datapath (even V only) pushes rel-err past the 2e-2 gate.

Ramp/tail scheduling (from NTFF trace analysis):
- DMA issue order is latency-driven: kx/qx chunk-0 and wk/wq interleave as
  the first transfers on the two hardware rings (sync + scalar) so the Q
  path does not queue behind the whole K/V stream; the DMA-completion
  semaphore pool is ~10 deep, so the first ten issues must be exactly the
  ten transfers slot (0,0) needs. Biases ride the scalar ring (the gpsimd
  software queue adds ~1.5us each); wo alone stays on gpsimd.
- PE warm-up dummies bridge the preamble-to-data window so the activity
  monitor never sees an idle PE and never halves the clock (a k=4 HAM
  window costs ~5us of doubled matmul time).
- Output path: Y tiles evict via the idle Pool engine (DVE copies were
  pacing the output projection at ~1.1us per 128-row block) and store as
  fp16 (halves the final-store tail; partial-sum fp16 quantization is
  ~6e-5, negligible against the 2e-2 gate).
- The last (chunk, head) slot is processed in two 256-query halves so the
  first half's output projection overlaps the second half's attention,
  shortening the serial tail to one half-chunk.
"""
import numpy as np
from contextlib import ExitStack

import ml_dtypes

import concourse.bacc as bacc
import concourse.mybir as mybir
import concourse.tile as tile
from concourse import bass_utils

# Problem shape (hardcoded per contract).
B, S, D, H, DH = 2, 2048, 768, 12, 64
NCORES = 8
NG = 4            # head groups
HG = H // NG      # heads per group (3)
G = HG * DH       # model dims per group (192)
SC = 512          # query-chunk length
NQ = S // SC      # 4 chunks
KB = 128          # key-block length
NKB = S // KB     # 16 blocks
KT6 = D // 128    # 6 contraction tiles for the projections
SEG = 2 * DH      # V segment width per head: 64 V columns + 64 ones columns
HC = 256          # query half-chunk (tail split of the last slot)
FP32 = mybir.dt.float32
F16 = mybir.dt.float16
CDT = mybir.dt.bfloat16   # matmul-operand dtype
NP_CDT = ml_dtypes.bfloat16

AF = mybir.ActivationFunctionType
ALU = mybir.AluOpType

_CACHE: dict = {}


def _build():
    nc = bacc.Bacc("TRN2", target_bir_lowering=False, debug=False)

    qT = nc.dram_tensor("qT", [NQ, 128, KT6, SC], CDT, kind="ExternalInput")
    kT = nc.dram_tensor("kT", [NQ, 128, KT6, SC], CDT, kind="ExternalInput")
    vT = nc.dram_tensor("vT", [NKB, 128, KT6, KB], CDT, kind="ExternalInput")
    # wk|wq|wv packed along the last axis: one wide tensor gives the weight
    # DMAs 3*G-wide rows (4.6KB), which win the per-packet round-robin
    # against the 3KB input rows on the other hardware ring — as three
    # separate tensors the weights crawled at ~100GB/s and the full-wk
    # dependency of the m0 eviction pushed the first logits out to ~18us.
    wkqv = nc.dram_tensor("wkqv", [128, KT6, 3 * G], CDT, kind="ExternalInput")
    wo = nc.dram_tensor("wo", [G, D], CDT, kind="ExternalInput")
    # all four bias halves in one [128, 4] fp32 transfer (cols: bk lo, bk
    # hi, bq lo, bq hi) — four separate tiny DMAs were eating early
    # completion-semaphore slots and serializing the critical qx issues.
    bias = nc.dram_tensor("bias", [128, 4], FP32, kind="ExternalInput")
    yp = nc.dram_tensor("yp", [S, D], F16, kind="ExternalOutput")

    with tile.TileContext(nc) as tc, ExitStack() as ctx:
        const = ctx.enter_context(tc.tile_pool(name="const", bufs=1))
        xin = ctx.enter_context(tc.tile_pool(name="xin", bufs=4))
        qxp = ctx.enter_context(tc.tile_pool(name="qxp", bufs=2))
        qtp = ctx.enter_context(tc.tile_pool(name="qtp", bufs=2))
        ppool = ctx.enter_context(tc.tile_pool(name="ppool", bufs=2))
        ctxp = ctx.enter_context(tc.tile_pool(name="ctxp", bufs=2))
        ypool = ctx.enter_context(tc.tile_pool(name="ypool", bufs=2))
        den = ctx.enter_context(tc.tile_pool(name="den", bufs=2))
        ps_proj = ctx.enter_context(tc.tile_pool(name="ps_proj", bufs=2, space="PSUM"))
        ps_log = ctx.enter_context(tc.tile_pool(name="ps_log", bufs=2, space="PSUM"))
        ps_ctx = ctx.enter_context(tc.tile_pool(name="ps_ctx", bufs=2, space="PSUM"))

        # ---- weights / constants ------------------------------------------
        # The first transfers on the two hardware DMA rings are exactly the
        # slot-(0,0) critical set, in completion-semaphore order (the pool
        # is ~10 deep and a blocked semaphore blocks the ISSUE): two wkqv
        # halves on the scalar ring, then kx c0 / qx c0 (two 3-t halves
        # each, matching the 3-t matmul units) on the sync ring, then the
        # single bias transfer. wo is not needed until the first output
        # projection (~45us) and rides the slow-but-idle gpsimd software
        # queue.
        wkqv_sb = const.tile([128, KT6, 3 * G], CDT)
        for j in range(2):
            nc.scalar.dma_start(
                wkqv_sb[:, 3 * j:3 * j + 3, :], wkqv.ap()[:, 3 * j:3 * j + 3, :]
            )

        # bf16 consts via fp32 memset + CAST-copy (uniform with f32r-era code).
        ones_f32 = const.tile([128, NKB], FP32)
        nc.vector.memset(ones_f32[:], 1.0)
        zero_f32 = const.tile([128, 1], FP32)
        nc.vector.memset(zero_f32[:], 0.0)

        def zero_fill(dst_ap, parts, cols):
            nc.vector.tensor_copy(
                dst_ap, zero_f32[0:parts, 0:1].to_broadcast((parts, cols))
            )

        # PE warm-up: the clock gate releases only after a sustained-busy
        # window, and the initial weight/input DMAs would otherwise leave
        # the PE idle; idling also makes the activity monitor clamp the
        # clock to half rate right as the first real matmuls issue. Burn
        # dependency-free matmuls on zeros; more dummies are interleaved
        # into the pre-stream projection units below to bridge arrival
        # jitter without ever idling the PE for the ~2us clamp threshold.
        wsrc = const.tile([128, SC], CDT)
        zero_fill(wsrc[:, :], 128, SC)
        wps = ps_log.tile([128, 2, SC], FP32, tag="pl", name="warmps")

        def dummy_mm(n):
            for _ in range(n):
                nc.tensor.matmul(
                    wps[:, 0, :], wsrc[:, 0:128], wsrc[:, :],
                    start=True, stop=True,
                )

        dummy_mm(8)

        # K^T per head, zero-padded to a full 128-partition contraction.
        # Partition placement matches the stacked Q^T tiles, so the padding
        # rows multiply zeros (or real rows multiply zero Q halves) and
        # every logits matmul runs with a full-height stationary — a
        # half-height (K=64) stationary makes the PE look half-idle to the
        # activity monitor, which then clamps the clock to half rate.
        # Only KTz0's pad is zeroed now; the other fills are deferred past
        # the first projections so the DVE queue can evict psum promptly.
        KTz0 = const.tile([128, S], CDT)   # [K_h0^T ; 0]
        KTz1 = const.tile([128, S], CDT)   # [0 ; K_h1^T]
        KTz2 = const.tile([128, S], CDT)   # [K_h2^T ; 0]
        zero_fill(KTz0[64:128, :], 64, S)
        # V blocks + 64 ones columns per head: the context matmul then
        # replicates the softmax denominator across 64 psum partitions
        # (stationary width is free), so normalization needs no
        # partition-broadcast DMA chain at all.
        Vg = const.tile([128, NKB, HG * SEG], CDT)

        def mk_fill_units():
            # Late const fills, packaged as filler units so they interleave
            # with slot (0,0)'s eviction traffic on the DVE queue. KTz2 is
            # first read in slot (0,2), Vg's ones in slot (0,1)'s context
            # matmuls, wo_sb1 at the first output projection.
            units = [
                lambda: zero_fill(KTz2[64:128, :], 64, S),
                lambda: zero_fill(wo_sb1[64:128, :], 64, D),
            ]
            for h in range(HG):
                units.append(lambda h=h: nc.vector.tensor_copy(
                    Vg[:, :, h * SEG + DH:(h + 1) * SEG],
                    ones_f32[:, 0:1, None].to_broadcast((128, NKB, DH)),
                ))
            return units

        mblocks = ((128, 0), (64, 128))  # (rows, row-offset) of the 192 dims

        # ---- K^T / V projections as emission units -----------------------
        # These are DMA-bound; instead of running them as serial phases
        # (PE half-idle, HAM re-throttles), they are spread as PE filler
        # into the first stream slots, hiding the loads under attention.
        def kt_units(c):
            kx = xin.tile([128, KT6, SC], CDT, tag="kx", name=f"kx_{c}")
            for j in range(2):
                nc.sync.dma_start(
                    kx[:, 3 * j:3 * j + 3, :], kT.ap()[c, :, 3 * j:3 * j + 3, :]
                )
            state = {}
            units = []

            def mk_mm(m, mp, mo, tpl):
                def emit():
                    if m not in state:
                        state[m] = ps_proj.tile(
                            [128, SC], FP32, tag="pp", name=f"ktps_{c}_{m}"
                        )
                    ps = state[m]
                    for t in tpl:
                        nc.tensor.matmul(
                            ps[:mp, :], wkqv_sb[:, t, mo:mo + mp], kx[:, t, :],
                            start=(t == 0), stop=(t == KT6 - 1),
                        )
                return emit

            def mk_evict(m, mp):
                def emit():
                    sl = slice(c * SC, (c + 1) * SC)
                    if m == 0:
                        nc.vector.tensor_scalar_add(
                            KTz0[0:64, sl], state[m][0:64, :],
                            bias_sb[0:64, 0:1],
                        )
                        nc.vector.tensor_scalar_add(
                            KTz1[64:128, sl], state[m][64:128, :],
                            bias_sb[64:128, 0:1],
                        )
                    else:
                        nc.vector.tensor_scalar_add(
                            KTz2[0:64, sl], state[m][0:64, :],
                            bias_sb[0:64, 1:2],
                        )
                return emit

            for m, (mp, mo) in enumerate(mblocks):
                for tpl in ((0, 1, 2), (3, 4, 5)):
                    units.append(mk_mm(m, mp, mo, tpl))
                units.append(mk_evict(m, mp))
            return units

        def v_units(sb):
            vx = xin.tile([128, KT6, KB], CDT, tag="vx", name=f"vx_{sb}")
            nc.sync.dma_start(vx[:], vT.ap()[sb])
            state = {}
            units = []

            def mk_mm(tpl, last):
                def emit():
                    if "ps" not in state:
                        state["ps"] = ps_proj.tile(
                            [128, G], FP32, tag="pp", name=f"vps_{sb}"
                        )
                    ps = state["ps"]
                    for t in tpl:
                        nc.tensor.matmul(
                            ps[:], vx[:, t, :], wkqv_sb[:, t, 2 * G:3 * G],
                            start=(t == 0), stop=(last and t == KT6 - 1),
                        )
                return emit

            def mk_evict():
                def emit():
                    # one strided copy lands all three heads' V columns
                    nc.vector.tensor_copy(
                        Vg[:, sb, :].rearrange(
                            "p (h s) -> p h s", h=HG
                        )[:, :, 0:DH],
                        state["ps"][:, :].rearrange(
                            "p (h s) -> p h s", h=HG
                        ),
                    )
                return emit

            units.append(mk_mm((0, 1, 2), False))
            units.append(mk_mm((3, 4, 5), True))
            units.append(mk_evict())
            return units

        # ---- phase 3: software-pipelined head stream ---------------------
        # Heads form one flat stream across chunks. Each slot interleaves
        # head i's logits+exp with head i-1's context matmuls so PE and ACT
        # both stay fed (in-order engines execute in emission order). The
        # normalization chain of head i-1 is emitted at slot end; the output
        # projection of a finished chunk is emitted one slot later, after
        # its normalization latency has been hidden under a full slot.
        QT = {}     # qc -> (QT0, QT1)
        CT = {}     # qc -> (ctxT0, ctxT1)

        KTZ = (KTz0, KTz1, KTz2)

        def head_slices(qc, h):
            qt0, qt1 = QT[qc]
            return KTZ[h], qt0 if h < 2 else qt1

        def qt_units(qc):
            # QT projection broken into emission units (PE filler). The qx
            # DMA and tile allocations happen now; matmuls are emitted as
            # the units are drained inside a kb2 loop.
            qx = qxp.tile([128, KT6, SC], CDT, tag="qx", name=f"qx_{qc}")
            for j in range(2):
                nc.sync.dma_start(
                    qx[:, 3 * j:3 * j + 3, :], qT.ap()[qc, :, 3 * j:3 * j + 3, :]
                )
            QT0 = qtp.tile([128, SC], CDT, tag="qt0", name=f"QT0_{qc}")
            QT1 = qtp.tile([128, SC], CDT, tag="qt1", name=f"QT1_{qc}")
            if qc < 2:
                # The qt1 pool has two buffers; their pad rows are zeroed on
                # first use and never overwritten (the m1 evict writes rows
                # 0:64 only), so chunks 2/3 reuse already-zeroed pads.
                zero_fill(QT1[64:128, :], 64, SC)
            QT[qc] = (QT0, QT1)
            units = []
            state = {}

            def mk_mm(m, mp, mo, tpl):
                def emit():
                    if m not in state:
                        state[m] = ps_proj.tile(
                            [128, SC], FP32, tag="pp", name=f"qtps_{qc}_{m}"
                        )
                    ps = state[m]
                    for t in tpl:
                        nc.tensor.matmul(
                            ps[:mp, :],
                            wkqv_sb[:, t, G + mo:G + mo + mp],
                            qx[:, t, :],
                            start=(t == 0), stop=(t == KT6 - 1),
                        )
                return emit

            def mk_evict(m, mp):
                def emit():
                    dst = QT0 if m == 0 else QT1
                    bcol = 2 if m == 0 else 3
                    nc.vector.tensor_scalar_add(
                        dst[0:mp, :], state[m][0:mp, :],
                        bias_sb[0:mp, bcol:bcol + 1],
                    )
                return emit

            for m, (mp, mo) in enumerate(mblocks):
                for tpl in ((0, 1, 2), (3, 4, 5)):
                    units.append(mk_mm(m, mp, mo, tpl))
                units.append(mk_evict(m, mp))
            return units

        def emit_norm(qc, h, pc, cs=slice(0, SC)):
            # Normalization: the denominator already sits replicated on psum
            # partitions DH..SEG (ones columns in Vg), so this is just a
            # wide approximate reciprocal plus a column-wise scale — three
            # DVE ops, no DMA hops. (The custom-DVE reciprocal must read
            # SBUF, not PSUM, hence the staging copy.)
            n = cs.stop - cs.start
            dsb = den.tile([64, SC], FP32, tag="dsb")
            nc.vector.tensor_copy(dsb[:, 0:n], pc[DH:SEG, cs])
            rbc = den.tile([64, SC], FP32, tag="rbc")
            nc.vector.reciprocal_approx_fast(rbc[:, 0:n], dsb[:, 0:n])
            ctxT0, ctxT1 = CT[qc]
            cdst = ctxT0[64 * h:64 * h + 64, cs] if h < 2 else ctxT1[0:64, cs]
            nc.vector.tensor_tensor(cdst, pc[0:DH, cs], rbc[:, 0:n], ALU.mult)

        def y_units(qc, alt_from=None):
            # Output projection as emission units (PE filler): 8 units of
            # two accumulating matmuls + psum eviction; each half-chunk's
            # DMA is split per 128-row block so the final store drains in
            # small pieces. The store is fp16 — half the bytes of the old
            # fp32 partials. Units >= alt_from alternate their psum
            # eviction between DVE and ACT (the tail units run after the
            # last exp, when ACT is idle, and DVE copies would otherwise
            # pace the serial drain at ~560ns per block).
            ctxT0, ctxT1 = CT[qc]
            ytiles = {}
            units = []

            def mk_unit(half, m, nh, on_act):
                def emit():
                    if half not in ytiles:
                        ytiles[half] = ypool.tile(
                            [128, 2, D], F16, tag="Y", name=f"Yt_{qc}_{half}"
                        )
                    Yt = ytiles[half]
                    sb = half * 2 + m
                    py = ps_proj.tile(
                        [128, D // 2], FP32, tag="pp", name=f"yps_{qc}_{sb}_{nh}"
                    )
                    nc.tensor.matmul(
                        py[:],
                        ctxT0[:, sb * 128:(sb + 1) * 128],
                        wo_sb0[:, nh * (D // 2):(nh + 1) * (D // 2)],
                        start=True, stop=False,
                    )
                    nc.tensor.matmul(
                        py[:],
                        ctxT1[:, sb * 128:(sb + 1) * 128],
                        wo_sb1[:, nh * (D // 2):(nh + 1) * (D // 2)],
                        start=False, stop=True,
                    )
                    ydst = Yt[:, m, nh * (D // 2):(nh + 1) * (D // 2)]
                    if on_act:
                        nc.scalar.activation(ydst, py[:], AF.Copy)
                    else:
                        nc.vector.tensor_copy(ydst, py[:])
                    if nh == 1:
                        r0 = qc * SC + half * 256 + m * 128
                        nc.sync.dma_start(
                            yp.ap()[r0:r0 + 128, :], Yt[:, m, :]
                        )
                return emit

            i = 0
            for half in range(2):
                for m in range(2):
                    for nh in range(2):
                        on_act = alt_from is not None and i >= alt_from and i % 2 == 1
                        units.append(mk_unit(half, m, nh, on_act))
                        i += 1
            return units

        def emit_y(qc):
            for u in y_units(qc):
                u()

        def emit_ctx_pair(prev, kb2, cs=slice(0, SC)):
            qc_p, h_p, P_p, pc_p = prev
            for j in range(2):
                kb = 2 * kb2 + j
                nc.tensor.matmul(
                    pc_p[0:SEG, cs],
                    Vg[:, kb, h_p * SEG:(h_p + 1) * SEG],
                    P_p[:, kb, cs],
                    start=(kb == 0), stop=(kb == NKB - 1),
                )

        stream = [(qc, h) for qc in range(NQ) for h in range(HG)]
        prev = None      # (qc, h, P, pc) of the head whose ctx is in flight

        # Slot (0,0) consumes only the m0 halves (heads 0/1 K, head-0/1 Q),
        # so just kt0.m0 + qt0.m0 run as the pre-stream block; every other
        # projection half streams in as filler placed one slot before its
        # first consumer. QT m0 feeds slot (qc,0) and QT m1 only (qc,2),
        # letting the QT prefetch straddle two otherwise-empty slots.
        KT_U = {0: kt_units(0)}
        QT_U = {0: qt_units(0)}
        # Deferred non-critical loads, emitted after the critical kx/qx
        # issues so they take later completion-semaphore slots. The tiles
        # are bound to the names the earlier-defined closures resolve at
        # call time (all unit emission happens after this point).
        bias_sb = const.tile([128, 4], FP32)
        nc.scalar.dma_start(bias_sb[:], bias.ap()[:, :])
        wo_sb0 = const.tile([128, D], CDT)
        nc.gpsimd.dma_start(wo_sb0[:], wo.ap()[0:128, :])
        wo_sb1 = const.tile([128, D], CDT)
        nc.gpsimd.dma_start(wo_sb1[0:64, :], wo.ap()[128:G, :])
        # m0 projections interleaved with dummy matmuls: each unit's data
        # arrives staggered (~0.5-1us apart), and the PE must never idle
        # for the ~2us activity-monitor threshold or the clock halves.
        KT_U[0][0]()
        dummy_mm(4)
        QT_U[0][0]()
        dummy_mm(4)
        KT_U[0][1]()
        dummy_mm(4)
        QT_U[0][1]()
        dummy_mm(4)
        KT_U[0][2]()
        QT_U[0][2]()
        dummy_mm(2)
        zero_fill(KTz1[0:64, :], 64, S)
        KT_U.update({c: kt_units(c) for c in (1, 2, 3)})

        NIT = NKB // 2
        for si, (qc, h) in enumerate(stream):
            last_slot = si == len(stream) - 1
            if h == 0:
                ctxT0_n = ctxp.tile([128, SC], CDT, tag="c0",
                                    name=f"ctxT0_{qc}")
                ctxT1_n = ctxp.tile([128, SC], CDT, tag="c1",
                                    name=f"ctxT1_{qc}")
                if qc < 2:
                    # two-buffer pool: pads zeroed on first use only (the
                    # norm writes rows 0:64; wo_sb1's zero pad rows guard
                    # the contraction anyway — this is belt-and-braces for
                    # Inf/NaN junk on a cold SBUF).
                    zero_fill(ctxT1_n[64:128, :], 64, SC)
                CT[qc] = (ctxT0_n, ctxT1_n)
            filler = []
            start_iter = 0
            if (qc, h) == (0, 0):
                filler.extend(KT_U[1][0:3])
                filler.extend(v_units(0) + v_units(1))
                filler.extend(KT_U[2][0:3])
                filler.extend(v_units(2) + v_units(3))
                filler.extend(KT_U[3][0:3])
                filler.extend(v_units(4) + v_units(5))
                filler.extend(mk_fill_units())
            elif (qc, h) == (0, 1):
                filler.extend(KT_U[0][3:6])
                filler.extend(QT_U[0][3:6])
                for sb in range(6, NKB):
                    filler.extend(v_units(sb))
            elif (qc, h) == (0, 2):
                for c in (1, 2, 3):
                    filler.extend(KT_U[c][3:6])
                QT_U[1] = qt_units(1)
                filler.extend(QT_U[1][0:3])
            elif h == 2 and qc + 1 < NQ:
                QT_U[qc + 1] = qt_units(qc + 1)
                filler.extend(QT_U[qc + 1][0:3])
            elif h == 0 and qc >= 1:
                filler = list(QT_U[qc][3:6])
            elif h == 1 and qc >= 1:
                filler = y_units(qc - 1)
                start_iter = 2

            kt_t, qt_t = head_slices(qc, h)

            if last_slot:
                # The last slot has no successor to hide its normalization
                # and output projection under, so it is processed in two
                # 256-query halves: half A's y-projection units become PE
                # filler for half B's attention, and only half B's
                # normalization + 8 y-units + 2 fp16 stores drain serially.
                P = ppool.tile([128, NKB, SC], CDT, tag="P")
                # y_units yields 8 units: 4 per 256-query half. Half A's
                # four units become PE filler during half B's attention
                # (their ctx columns 0:256 are normalized by then); half
                # B's four drain after its own normalization.
                yu = y_units(qc, alt_from=4)
                for half in range(2):
                    cs = slice(half * HC, (half + 1) * HC)
                    pc_h = ps_ctx.tile([128, SC], FP32, tag="pc")
                    hfill = yu[0:4] if half == 1 else []
                    for kb2 in range(NIT):
                        pl = ps_log.tile([128, 2, SC], FP32, tag="pl")
                        for j in range(2):
                            kb = 2 * kb2 + j
                            nc.tensor.matmul(
                                pl[:, j, cs],
                                kt_t[:, kb * KB:(kb + 1) * KB],
                                qt_t[:, cs],
                                start=True, stop=True,
                            )
                        nc.scalar.activation(
                            P[:, 2 * kb2:2 * kb2 + 2, cs], pl[:, :, cs],
                            AF.Exp, scale=1.0 / np.sqrt(DH)
                        )
                        if half == 0 and prev is not None:
                            emit_ctx_pair(prev, kb2)
                        if hfill:
                            n = -(-len(hfill) // (NIT - kb2))
                            for _ in range(n):
                                hfill.pop(0)()
                        if kb2 >= 1:
                            emit_ctx_pair((qc, h, P, pc_h), kb2 - 1, cs)
                    emit_ctx_pair((qc, h, P, pc_h), NIT - 1, cs)
                    if half == 0 and prev is not None:
                        emit_norm(prev[0], prev[1], prev[3])
                    emit_norm(qc, h, pc_h, cs)
                # keep the PE busy through the final normalization latency
                # so the activity monitor does not clamp the clock.
                dps = ps_log.tile([128, 2, SC], FP32, tag="pl", name="drainps")
                for _ in range(6):
                    nc.tensor.matmul(
                        dps[:, 0, :], wsrc[:, 0:128], wsrc[:, :],
                        start=True, stop=True,
                    )
                for u in yu[4:8]:
                    u()
                break

            P = ppool.tile([128, NKB, SC], CDT, tag="P")
            for kb2 in range(NIT):
                pl = ps_log.tile([128, 2, SC], FP32, tag="pl")
                for j in range(2):
                    kb = 2 * kb2 + j
                    nc.tensor.matmul(
                        pl[:, j, :],
                        kt_t[:, kb * KB:(kb + 1) * KB],
                        qt_t[:, :],
                        start=True, stop=True,
                    )
                nc.scalar.activation(
                    P[:, 2 * kb2:2 * kb2 + 2, :], pl[:],
                    AF.Exp, scale=1.0 / np.sqrt(DH)
                )
                if filler and kb2 >= start_iter:
                    n = -(-len(filler) // (NIT - kb2))
                    for _ in range(n):
                        filler.pop(0)()
                if prev is not None:
                    emit_ctx_pair(prev, kb2)
            if prev is not None:
                emit_norm(prev[0], prev[1], prev[3])
            for u in filler:
                u()
            pc_cur = ps_ctx.tile([128, SC], FP32, tag="pc")
            prev = (qc, h, P, pc_cur)

    nc.compile()
    return nc


def _get_nc():
    if "nc" not in _CACHE:
        _CACHE["nc"] = _build()
    return _CACHE["nc"]


def _tile_x(xb, chunk):
    # x (S, D) -> x^T tiled (S/chunk, 128, KT6, chunk), contiguous, bf16
    xt = np.asarray(xb, dtype=np.float32).T.astype(NP_CDT)
    return np.ascontiguousarray(
        xt.reshape(KT6, 128, S // chunk, chunk).transpose(2, 1, 0, 3)
    )


def _tile_w(w):
    # (D, G) -> (128, KT6, G) contiguous, bf16
    w = np.asarray(w, dtype=np.float32).astype(NP_CDT)
    return np.ascontiguousarray(w.reshape(KT6, 128, G).transpose(1, 0, 2))


def _in_maps(v, k, q, wq, bq, wk, bk, wv, bv, wo, bo):
    bf16 = lambda a: np.ascontiguousarray(
        np.asarray(a, dtype=np.float32).astype(NP_CDT)
    )
    qTb = [_tile_x(q[b], SC) for b in range(B)]
    kTb = [_tile_x(k[b], SC) for b in range(B)]
    vTb = [_tile_x(v[b], KB) for b in range(B)]
    maps = []
    for c in range(NCORES):
        b, g = divmod(c, NG)
        cols = slice(g * G, (g + 1) * G)
        # wk|wq|wv side by side per t-tile (one wide, RR-friendly DMA).
        wkqv = np.ascontiguousarray(np.concatenate(
            [_tile_w(np.asarray(w)[:, cols]) for w in (wk, wq, wv)], axis=2
        ))
        # bias halves packed as columns: bk lo, bk hi, bq lo, bq hi.
        bp = np.zeros((128, 4), dtype=np.float32)
        bp[:, 0] = np.asarray(bk, dtype=np.float32)[cols][0:128]
        bp[0:64, 1] = np.asarray(bk, dtype=np.float32)[cols][128:G]
        bp[:, 2] = np.asarray(bq, dtype=np.float32)[cols][0:128]
        bp[0:64, 3] = np.asarray(bq, dtype=np.float32)[cols][128:G]
        maps.append({
            "qT": qTb[b],
            "kT": kTb[b],
            "vT": vTb[b],
            "wkqv": wkqv,
            "wo": bf16(wo[cols, :]),
            "bias": bp,
        })
    return maps


def kernel(v, k, q, wq, bq, wk, bk, wv, bv, wo, bo, _trace=False):
    nc = _get_nc()
    in_maps = _in_maps(v, k, q, wq, bq, wk, bk, wv, bv, wo, bo)
    res = bass_utils.run_bass_kernel_spmd(
        nc, in_maps, core_ids=list(range(NCORES)), trace=_trace
    )
    # softmax weights sum to 1, so the V bias shifts ctx by exactly bv;
    # its contribution to the output is the constant row bv @ wo + bo.
    corr = (np.asarray(bv, dtype=np.float64) @ np.asarray(wo, dtype=np.float64)
            + np.asarray(bo, dtype=np.float64)).astype(np.float32)
    out = np.empty((B, S, D), dtype=np.float32)
    for b in range(B):
        acc = res.results[4 * b]["yp"].astype(np.float32)
        for g in range(1, NG):
            acc = acc + res.results[4 * b + g]["yp"].astype(np.float32)
        out[b] = acc + corr[None, :]
    if _trace:
        kernel.last_result = res
    return out


# revision 28
# speedup vs baseline: 1.0356x; 1.0183x over previous
"""Multi-head self-attention (B=2, S=2048, D=768, H=12) on 8 trn2 NeuronCores.

Sharding: core c = 4*b + g handles batch b and head-group g (3 heads = 192 of
the 768 model dims). Weights are column-split (wq/wk/wv) and row-split (wo);
each core emits a partial (2048, 768) output; the host sums the 4 group
partials per batch and adds bo.

Device-side dataflow is transpose-free: inputs arrive pre-transposed (D, S),
so projections produce Q^T/K^T in (head_dim, S) layout which feeds the
logits matmul directly; softmax is computed as exp(logits/8) without
max-subtraction (logits are ~N(0,1), exp cannot overflow) with denominators
obtained from a ones-column appended to V in the context matmul; the context
comes out transposed (dims, S), which is exactly the stationary operand the
output projection needs.

Matmul operands are bfloat16 (1 cyc/row streaming, half the DMA/SBUF bytes
of fp32); accumulation stays fp32 in PSUM. Inputs/weights are converted to
bf16 on the host. fp8 was measured and rejected: e4m3 anywhere in the
datapath (even V only) pushes rel-err past the 2e-2 gate.

Ramp/tail scheduling (from NTFF trace analysis):
- DMA issue order is latency-driven: kx/qx chunk-0 and wk/wq interleave as
  the first transfers on the two hardware rings (sync + scalar) so the Q
  path does not queue behind the whole K/V stream; the DMA-completion
  semaphore pool is ~10 deep, so the first ten issues must be exactly the
  ten transfers slot (0,0) needs. Biases ride the scalar ring (the gpsimd
  software queue adds ~1.5us each); wo alone stays on gpsimd.
- PE warm-up dummies bridge the preamble-to-data window so the activity
  monitor never sees an idle PE and never halves the clock (a k=4 HAM
  window costs ~5us of doubled matmul time).
- Output path: Y tiles evict via the idle Pool engine (DVE copies were
  pacing the output projection at ~1.1us per 128-row block) and store as
  fp16 (halves the final-store tail; partial-sum fp16 quantization is
  ~6e-5, negligible against the 2e-2 gate).
- The last (chunk, head) slot is processed in two 256-query halves so the
  first half's output projection overlaps the second half's attention,
  shortening the serial tail to one half-chunk.
"""
import numpy as np
from contextlib import ExitStack

import ml_dtypes

import concourse.bacc as bacc
import concourse.mybir as mybir
import concourse.tile as tile
from concourse import bass_utils

# Problem shape (hardcoded per contract).
B, S, D, H, DH = 2, 2048, 768, 12, 64
NCORES = 8
NG = 4            # head groups
HG = H // NG      # heads per group (3)
G = HG * DH       # model dims per group (192)
SC = 512          # query-chunk length
NQ = S // SC      # 4 chunks
KB = 128          # key-block length
NKB = S // KB     # 16 blocks
KT6 = D // 128    # 6 contraction tiles for the projections
SEG = 2 * DH      # V segment width per head: 64 V columns + 64 ones columns
HC = 256          # query half-chunk (tail split of the last slot)
FP32 = mybir.dt.float32
F16 = mybir.dt.float16
CDT = mybir.dt.bfloat16   # matmul-operand dtype
NP_CDT = ml_dtypes.bfloat16

AF = mybir.ActivationFunctionType
ALU = mybir.AluOpType

_CACHE: dict = {}


def _build():
    nc = bacc.Bacc("TRN2", target_bir_lowering=False, debug=False)

    qT = nc.dram_tensor("qT", [NQ, 128, KT6, SC], CDT, kind="ExternalInput")
    kT = nc.dram_tensor("kT", [NQ, 128, KT6, SC], CDT, kind="ExternalInput")
    vT = nc.dram_tensor("vT", [NKB, 128, KT6, KB], CDT, kind="ExternalInput")
    # wk|wq packed along the last axis: one wide tensor gives the weight
    # DMAs 2*G-wide rows, which hold up in the per-packet round-robin
    # against the 3KB input rows on the other hardware ring — as three
    # separate tensors the weights crawled at ~100GB/s and the full-wk
    # dependency of the m0 eviction pushed the first logits out to ~18us.
    # wv rides separately after the critical set (first needed ~20us).
    wkq = nc.dram_tensor("wkq", [128, KT6, 2 * G], CDT, kind="ExternalInput")
    wv = nc.dram_tensor("wv", [128, KT6, G], CDT, kind="ExternalInput")
    wo = nc.dram_tensor("wo", [G, D], CDT, kind="ExternalInput")
    # all four bias halves in one [128, 4] fp32 transfer (cols: bk lo, bk
    # hi, bq lo, bq hi) — four separate tiny DMAs were eating early
    # completion-semaphore slots and serializing the critical qx issues.
    bias = nc.dram_tensor("bias", [128, 4], FP32, kind="ExternalInput")
    yp = nc.dram_tensor("yp", [S, D], F16, kind="ExternalOutput")

    with tile.TileContext(nc) as tc, ExitStack() as ctx:
        const = ctx.enter_context(tc.tile_pool(name="const", bufs=1))
        xin = ctx.enter_context(tc.tile_pool(name="xin", bufs=4))
        qxp = ctx.enter_context(tc.tile_pool(name="qxp", bufs=2))
        qtp = ctx.enter_context(tc.tile_pool(name="qtp", bufs=2))
        ppool = ctx.enter_context(tc.tile_pool(name="ppool", bufs=2))
        ctxp = ctx.enter_context(tc.tile_pool(name="ctxp", bufs=2))
        ypool = ctx.enter_context(tc.tile_pool(name="ypool", bufs=2))
        den = ctx.enter_context(tc.tile_pool(name="den", bufs=2))
        ps_proj = ctx.enter_context(tc.tile_pool(name="ps_proj", bufs=2, space="PSUM"))
        ps_log = ctx.enter_context(tc.tile_pool(name="ps_log", bufs=2, space="PSUM"))
        ps_ctx = ctx.enter_context(tc.tile_pool(name="ps_ctx", bufs=2, space="PSUM"))

        # ---- weights / constants ------------------------------------------
        # The first transfers on the two hardware DMA rings are exactly the
        # slot-(0,0) critical set, in completion-semaphore order (the pool
        # is ~10 deep and a blocked semaphore blocks the ISSUE): two wkqv
        # halves on the scalar ring, then kx c0 / qx c0 (two 3-t halves
        # each, matching the 3-t matmul units) on the sync ring, then the
        # single bias transfer. wo is not needed until the first output
        # projection (~45us) and rides the slow-but-idle gpsimd software
        # queue.
        wkq_sb = const.tile([128, KT6, 2 * G], CDT)
        for j in range(2):
            nc.scalar.dma_start(
                wkq_sb[:, 3 * j:3 * j + 3, :], wkq.ap()[:, 3 * j:3 * j + 3, :]
            )

        # bf16 consts via fp32 memset + CAST-copy (uniform with f32r-era code).
        ones_f32 = const.tile([128, NKB], FP32)
        nc.vector.memset(ones_f32[:], 1.0)
        zero_f32 = const.tile([128, 1], FP32)
        nc.vector.memset(zero_f32[:], 0.0)

        def zero_fill(dst_ap, parts, cols):
            nc.vector.tensor_copy(
                dst_ap, zero_f32[0:parts, 0:1].to_broadcast((parts, cols))
            )

        # PE warm-up: the clock gate releases only after a sustained-busy
        # window, and the initial weight/input DMAs would otherwise leave
        # the PE idle; idling also makes the activity monitor clamp the
        # clock to half rate right as the first real matmuls issue. Burn
        # dependency-free matmuls on zeros; more dummies are interleaved
        # into the pre-stream projection units below to bridge arrival
        # jitter without ever idling the PE for the ~2us clamp threshold.
        wsrc = const.tile([128, SC], CDT)
        zero_fill(wsrc[:, :], 128, SC)
        wps = ps_log.tile([128, 2, SC], FP32, tag="pl", name="warmps")

        def dummy_mm(n):
            for _ in range(n):
                nc.tensor.matmul(
                    wps[:, 0, :], wsrc[:, 0:128], wsrc[:, :],
                    start=True, stop=True,
                )

        # K^T per head, zero-padded to a full 128-partition contraction.
        # Partition placement matches the stacked Q^T tiles, so the padding
        # rows multiply zeros (or real rows multiply zero Q halves) and
        # every logits matmul runs with a full-height stationary — a
        # half-height (K=64) stationary makes the PE look half-idle to the
        # activity monitor, which then clamps the clock to half rate.
        # Only KTz0's pad is zeroed now; the other fills are deferred past
        # the first projections so the DVE queue can evict psum promptly.
        KTz0 = const.tile([128, S], CDT)   # [K_h0^T ; 0]
        KTz1 = const.tile([128, S], CDT)   # [0 ; K_h1^T]
        KTz2 = const.tile([128, S], CDT)   # [K_h2^T ; 0]
        zero_fill(KTz0[64:128, :], 64, S)
        # V blocks + 64 ones columns per head: the context matmul then
        # replicates the softmax denominator across 64 psum partitions
        # (stationary width is free), so normalization needs no
        # partition-broadcast DMA chain at all.
        Vg = const.tile([128, NKB, HG * SEG], CDT)

        def mk_fill_units():
            # Late const fills, packaged as filler units so they interleave
            # with slot (0,0)'s eviction traffic on the DVE queue. KTz2 is
            # first read in slot (0,2), Vg's ones in slot (0,1)'s context
            # matmuls, wo_sb1 at the first output projection.
            units = [
                lambda: zero_fill(KTz2[64:128, :], 64, S),
                lambda: zero_fill(wo_sb1[64:128, :], 64, D),
            ]
            for h in range(HG):
                units.append(lambda h=h: nc.vector.tensor_copy(
                    Vg[:, :, h * SEG + DH:(h + 1) * SEG],
                    ones_f32[:, 0:1, None].to_broadcast((128, NKB, DH)),
                ))
            return units

        mblocks = ((128, 0), (64, 128))  # (rows, row-offset) of the 192 dims

        # ---- K^T / V projections as emission units -----------------------
        # These are DMA-bound; instead of running them as serial phases
        # (PE half-idle, HAM re-throttles), they are spread as PE filler
        # into the first stream slots, hiding the loads under attention.
        def kt_units(c, kx=None):
            if kx is None:
                kx = xin.tile([128, KT6, SC], CDT, tag="kx", name=f"kx_{c}")
                for j in range(2):
                    nc.sync.dma_start(
                        kx[:, 3 * j:3 * j + 3, :],
                        kT.ap()[c, :, 3 * j:3 * j + 3, :],
                    )
            state = {}
            units = []

            def mk_mm(m, mp, mo, tpl):
                def emit():
                    if m not in state:
                        state[m] = ps_proj.tile(
                            [128, SC], FP32, tag="pp", name=f"ktps_{c}_{m}"
                        )
                    ps = state[m]
                    for t in tpl:
                        nc.tensor.matmul(
                            ps[:mp, :], wkq_sb[:, t, mo:mo + mp], kx[:, t, :],
                            start=(t == 0), stop=(t == KT6 - 1),
                        )
                return emit

            def mk_evict(m, mp):
                def emit():
                    sl = slice(c * SC, (c + 1) * SC)
                    if m == 0:
                        nc.vector.tensor_scalar_add(
                            KTz0[0:64, sl], state[m][0:64, :],
                            bias_sb[0:64, 0:1],
                        )
                        nc.vector.tensor_scalar_add(
                            KTz1[64:128, sl], state[m][64:128, :],
                            bias_sb[64:128, 0:1],
                        )
                    else:
                        nc.vector.tensor_scalar_add(
                            KTz2[0:64, sl], state[m][0:64, :],
                            bias_sb[0:64, 1:2],
                        )
                return emit

            for m, (mp, mo) in enumerate(mblocks):
                for tpl in ((0, 1, 2), (3, 4, 5)):
                    units.append(mk_mm(m, mp, mo, tpl))
                units.append(mk_evict(m, mp))
            return units

        def v_units(sb):
            vx = xin.tile([128, KT6, KB], CDT, tag="vx", name=f"vx_{sb}")
            nc.sync.dma_start(vx[:], vT.ap()[sb])
            state = {}
            units = []

            def mk_mm(tpl, last):
                def emit():
                    if "ps" not in state:
                        state["ps"] = ps_proj.tile(
                            [128, G], FP32, tag="pp", name=f"vps_{sb}"
                        )
                    ps = state["ps"]
                    for t in tpl:
                        nc.tensor.matmul(
                            ps[:], vx[:, t, :], wv_sb[:, t, :],
                            start=(t == 0), stop=(last and t == KT6 - 1),
                        )
                return emit

            def mk_evict():
                def emit():
                    # one strided copy lands all three heads' V columns
                    nc.vector.tensor_copy(
                        Vg[:, sb, :].rearrange(
                            "p (h s) -> p h s", h=HG
                        )[:, :, 0:DH],
                        state["ps"][:, :].rearrange(
                            "p (h s) -> p h s", h=HG
                        ),
                    )
                return emit

            units.append(mk_mm((0, 1, 2), False))
            units.append(mk_mm((3, 4, 5), True))
            units.append(mk_evict())
            return units

        # ---- phase 3: software-pipelined head stream ---------------------
        # Heads form one flat stream across chunks. Each slot interleaves
        # head i's logits+exp with head i-1's context matmuls so PE and ACT
        # both stay fed (in-order engines execute in emission order). The
        # normalization chain of head i-1 is emitted at slot end; the output
        # projection of a finished chunk is emitted one slot later, after
        # its normalization latency has been hidden under a full slot.
        QT = {}     # qc -> (QT0, QT1)
        CT = {}     # qc -> (ctxT0, ctxT1)

        KTZ = (KTz0, KTz1, KTz2)

        def head_slices(qc, h):
            qt0, qt1 = QT[qc]
            return KTZ[h], qt0 if h < 2 else qt1

        def qt_units(qc, qx=None):
            # QT projection broken into emission units (PE filler). The qx
            # DMA and tile allocations happen now; matmuls are emitted as
            # the units are drained inside a kb2 loop.
            if qx is None:
                qx = qxp.tile([128, KT6, SC], CDT, tag="qx", name=f"qx_{qc}")
                for j in range(2):
                    nc.sync.dma_start(
                        qx[:, 3 * j:3 * j + 3, :],
                        qT.ap()[qc, :, 3 * j:3 * j + 3, :],
                    )
            QT0 = qtp.tile([128, SC], CDT, tag="qt0", name=f"QT0_{qc}")
            QT1 = qtp.tile([128, SC], CDT, tag="qt1", name=f"QT1_{qc}")
            if qc < 2:
                # The qt1 pool has two buffers; their pad rows are zeroed on
                # first use and never overwritten (the m1 evict writes rows
                # 0:64 only), so chunks 2/3 reuse already-zeroed pads.
                zero_fill(QT1[64:128, :], 64, SC)
            QT[qc] = (QT0, QT1)
            units = []
            state = {}

            def mk_mm(m, mp, mo, tpl):
                def emit():
                    if m not in state:
                        state[m] = ps_proj.tile(
                            [128, SC], FP32, tag="pp", name=f"qtps_{qc}_{m}"
                        )
                    ps = state[m]
                    for t in tpl:
                        nc.tensor.matmul(
                            ps[:mp, :],
                            wkq_sb[:, t, G + mo:G + mo + mp],
                            qx[:, t, :],
                            start=(t == 0), stop=(t == KT6 - 1),
                        )
                return emit

            def mk_evict(m, mp):
                def emit():
                    dst = QT0 if m == 0 else QT1
                    bcol = 2 if m == 0 else 3
                    nc.vector.tensor_scalar_add(
                        dst[0:mp, :], state[m][0:mp, :],
                        bias_sb[0:mp, bcol:bcol + 1],
                    )
                return emit

            for m, (mp, mo) in enumerate(mblocks):
                for tpl in ((0, 1, 2), (3, 4, 5)):
                    units.append(mk_mm(m, mp, mo, tpl))
                units.append(mk_evict(m, mp))
            return units

        def emit_norm(qc, h, pc, cs=slice(0, SC)):
            # Normalization: the denominator already sits replicated on psum
            # partitions DH..SEG (ones columns in Vg), so this is just a
            # wide approximate reciprocal plus a column-wise scale — three
            # DVE ops, no DMA hops. (The custom-DVE reciprocal must read
            # SBUF, not PSUM, hence the staging copy.)
            n = cs.stop - cs.start
            dsb = den.tile([64, SC], FP32, tag="dsb")
            nc.vector.tensor_copy(dsb[:, 0:n], pc[DH:SEG, cs])
            rbc = den.tile([64, SC], FP32, tag="rbc")
            nc.vector.reciprocal_approx_fast(rbc[:, 0:n], dsb[:, 0:n])
            ctxT0, ctxT1 = CT[qc]
            cdst = ctxT0[64 * h:64 * h + 64, cs] if h < 2 else ctxT1[0:64, cs]
            nc.vector.tensor_tensor(cdst, pc[0:DH, cs], rbc[:, 0:n], ALU.mult)

        def y_units(qc, alt_from=None):
            # Output projection as emission units (PE filler): 8 units of
            # two accumulating matmuls + psum eviction; each half-chunk's
            # DMA is split per 128-row block so the final store drains in
            # small pieces. The store is fp16 — half the bytes of the old
            # fp32 partials. Units >= alt_from alternate their psum
            # eviction between DVE and ACT (the tail units run after the
            # last exp, when ACT is idle, and DVE copies would otherwise
            # pace the serial drain at ~560ns per block).
            ctxT0, ctxT1 = CT[qc]
            ytiles = {}
            units = []

            def mk_unit(half, m, nh, on_act):
                def emit():
                    if half not in ytiles:
                        ytiles[half] = ypool.tile(
                            [128, 2, D], F16, tag="Y", name=f"Yt_{qc}_{half}"
                        )
                    Yt = ytiles[half]
                    sb = half * 2 + m
                    py = ps_proj.tile(
                        [128, D // 2], FP32, tag="pp", name=f"yps_{qc}_{sb}_{nh}"
                    )
                    nc.tensor.matmul(
                        py[:],
                        ctxT0[:, sb * 128:(sb + 1) * 128],
                        wo_sb0[:, nh * (D // 2):(nh + 1) * (D // 2)],
                        start=True, stop=False,
                    )
                    nc.tensor.matmul(
                        py[:],
                        ctxT1[:, sb * 128:(sb + 1) * 128],
                        wo_sb1[:, nh * (D // 2):(nh + 1) * (D // 2)],
                        start=False, stop=True,
                    )
                    ydst = Yt[:, m, nh * (D // 2):(nh + 1) * (D // 2)]
                    if on_act:
                        nc.scalar.activation(ydst, py[:], AF.Copy)
                    else:
                        nc.vector.tensor_copy(ydst, py[:])
                    if nh == 1:
                        r0 = qc * SC + half * 256 + m * 128
                        nc.sync.dma_start(
                            yp.ap()[r0:r0 + 128, :], Yt[:, m, :]
                        )
                return emit

            i = 0
            for half in range(2):
                for m in range(2):
                    for nh in range(2):
                        on_act = alt_from is not None and i >= alt_from and i % 2 == 1
                        units.append(mk_unit(half, m, nh, on_act))
                        i += 1
            return units

        def emit_y(qc):
            for u in y_units(qc):
                u()

        def emit_ctx_pair(prev, kb2, cs=slice(0, SC)):
            qc_p, h_p, P_p, pc_p = prev
            for j in range(2):
                kb = 2 * kb2 + j
                nc.tensor.matmul(
                    pc_p[0:SEG, cs],
                    Vg[:, kb, h_p * SEG:(h_p + 1) * SEG],
                    P_p[:, kb, cs],
                    start=(kb == 0), stop=(kb == NKB - 1),
                )

        stream = [(qc, h) for qc in range(NQ) for h in range(HG)]
        prev = None      # (qc, h, P, pc) of the head whose ctx is in flight

        # Slot (0,0) consumes only the m0 halves (heads 0/1 K, head-0/1 Q),
        # so just kt0.m0 + qt0.m0 run as the pre-stream block; every other
        # projection half streams in as filler placed one slot before its
        # first consumer. QT m0 feeds slot (qc,0) and QT m1 only (qc,2),
        # letting the QT prefetch straddle two otherwise-empty slots.
        # Chunk-0 input DMAs interleave Q-first in 3-t halves: the Q path
        # has no consumer besides slot (0,0) while the K path also feeds
        # kb>=4 via the c1-3 fillers, so Q going first balances the two
        # eviction chains.
        kx0 = xin.tile([128, KT6, SC], CDT, tag="kx", name="kx_0")
        qx0 = qxp.tile([128, KT6, SC], CDT, tag="qx", name="qx_0")
        for j in range(2):
            sl = slice(3 * j, 3 * j + 3)
            nc.sync.dma_start(qx0[:, sl, :], qT.ap()[0, :, sl, :])
            nc.sync.dma_start(kx0[:, sl, :], kT.ap()[0, :, sl, :])
        # Initial PE warm-up: ~4us of gap-free dummies earns the activity
        # monitor's full-clock grant (~3.5us of uninterrupted busy) right
        # as the first input slices land.
        dummy_mm(10)
        KT_U = {0: kt_units(0, kx=kx0)}
        QT_U = {0: qt_units(0, qx=qx0)}
        # Deferred non-critical loads, emitted after the critical kx/qx
        # issues so they take later completion-semaphore slots. The tiles
        # are bound to the names the earlier-defined closures resolve at
        # call time (all unit emission happens after this point).
        bias_sb = const.tile([128, 4], FP32)
        nc.scalar.dma_start(bias_sb[:], bias.ap()[:, :])
        wv_sb = const.tile([128, KT6, G], CDT)
        nc.scalar.dma_start(wv_sb[:], wv.ap()[:, :, :])
        wo_sb0 = const.tile([128, D], CDT)
        nc.gpsimd.dma_start(wo_sb0[:], wo.ap()[0:128, :])
        wo_sb1 = const.tile([128, D], CDT)
        nc.gpsimd.dma_start(wo_sb1[0:64, :], wo.ap()[128:G, :])
        # m0 projections interleaved with dummy matmuls: each unit's data
        # arrives staggered (~0.5-1us apart), and the PE must never idle
        # for the ~1us activity-monitor threshold or the clock halves.
        QT_U[0][0]()
        dummy_mm(3)
        KT_U[0][0]()
        dummy_mm(3)
        QT_U[0][1]()
        dummy_mm(3)
        KT_U[0][1]()
        dummy_mm(3)
        QT_U[0][2]()
        KT_U[0][2]()
        dummy_mm(3)
        zero_fill(KTz1[0:64, :], 64, S)
        KT_U.update({c: kt_units(c) for c in (1, 2, 3)})

        NIT = NKB // 2
        for si, (qc, h) in enumerate(stream):
            last_slot = si == len(stream) - 1
            if h == 0:
                ctxT0_n = ctxp.tile([128, SC], CDT, tag="c0",
                                    name=f"ctxT0_{qc}")
                ctxT1_n = ctxp.tile([128, SC], CDT, tag="c1",
                                    name=f"ctxT1_{qc}")
                if qc < 2:
                    # two-buffer pool: pads zeroed on first use only (the
                    # norm writes rows 0:64; wo_sb1's zero pad rows guard
                    # the contraction anyway — this is belt-and-braces for
                    # Inf/NaN junk on a cold SBUF).
                    zero_fill(ctxT1_n[64:128, :], 64, SC)
                CT[qc] = (ctxT0_n, ctxT1_n)
            filler = []
            start_iter = 0
            if (qc, h) == (0, 0):
                filler.extend(KT_U[1][0:3])
                filler.extend(v_units(0) + v_units(1))
                filler.extend(KT_U[2][0:3])
                filler.extend(v_units(2) + v_units(3))
                filler.extend(KT_U[3][0:3])
                filler.extend(v_units(4) + v_units(5))
                filler.extend(mk_fill_units())
            elif (qc, h) == (0, 1):
                filler.extend(KT_U[0][3:6])
                filler.extend(QT_U[0][3:6])
                for sb in range(6, NKB):
                    filler.extend(v_units(sb))
            elif (qc, h) == (0, 2):
                for c in (1, 2, 3):
                    filler.extend(KT_U[c][3:6])
                QT_U[1] = qt_units(1)
                filler.extend(QT_U[1][0:3])
            elif h == 2 and qc + 1 < NQ:
                QT_U[qc + 1] = qt_units(qc + 1)
                filler.extend(QT_U[qc + 1][0:3])
            elif h == 0 and qc >= 1:
                filler = list(QT_U[qc][3:6])
            elif h == 1 and qc >= 1:
                filler = y_units(qc - 1)
                start_iter = 2

            kt_t, qt_t = head_slices(qc, h)

            if last_slot:
                # The last slot has no successor to hide its normalization
                # and output projection under, so it is processed in two
                # 256-query halves: half A's y-projection units become PE
                # filler for half B's attention, and only half B's
                # normalization + 8 y-units + 2 fp16 stores drain serially.
                P = ppool.tile([128, NKB, SC], CDT, tag="P")
                # y_units yields 8 units: 4 per 256-query half. Half A's
                # four units become PE filler during half B's attention
                # (their ctx columns 0:256 are normalized by then); half
                # B's four drain after its own normalization.
                yu = y_units(qc, alt_from=4)
                for half in range(2):
                    cs = slice(half * HC, (half + 1) * HC)
                    pc_h = ps_ctx.tile([128, SC], FP32, tag="pc")
                    hfill = yu[0:4] if half == 1 else []
                    for kb2 in range(NIT):
                        pl = ps_log.tile([128, 2, SC], FP32, tag="pl")
                        for j in range(2):
                            kb = 2 * kb2 + j
                            nc.tensor.matmul(
                                pl[:, j, cs],
                                kt_t[:, kb * KB:(kb + 1) * KB],
                                qt_t[:, cs],
                                start=True, stop=True,
                            )
                        nc.scalar.activation(
                            P[:, 2 * kb2:2 * kb2 + 2, cs], pl[:, :, cs],
                            AF.Exp, scale=1.0 / np.sqrt(DH)
                        )
                        if half == 0 and prev is not None:
                            emit_ctx_pair(prev, kb2)
                        if hfill:
                            n = -(-len(hfill) // (NIT - kb2))
                            for _ in range(n):
                                hfill.pop(0)()
                        if kb2 >= 1:
                            emit_ctx_pair((qc, h, P, pc_h), kb2 - 1, cs)
                    emit_ctx_pair((qc, h, P, pc_h), NIT - 1, cs)
                    if half == 0 and prev is not None:
                        emit_norm(prev[0], prev[1], prev[3])
                    emit_norm(qc, h, pc_h, cs)
                # keep the PE busy through the final normalization latency
                # so the activity monitor does not clamp the clock.
                dps = ps_log.tile([128, 2, SC], FP32, tag="pl", name="drainps")
                for _ in range(6):
                    nc.tensor.matmul(
                        dps[:, 0, :], wsrc[:, 0:128], wsrc[:, :],
                        start=True, stop=True,
                    )
                for u in yu[4:8]:
                    u()
                break

            P = ppool.tile([128, NKB, SC], CDT, tag="P")
            for kb2 in range(NIT):
                pl = ps_log.tile([128, 2, SC], FP32, tag="pl")
                for j in range(2):
                    kb = 2 * kb2 + j
                    nc.tensor.matmul(
                        pl[:, j, :],
                        kt_t[:, kb * KB:(kb + 1) * KB],
                        qt_t[:, :],
                        start=True, stop=True,
                    )
                nc.scalar.activation(
                    P[:, 2 * kb2:2 * kb2 + 2, :], pl[:],
                    AF.Exp, scale=1.0 / np.sqrt(DH)
                )
                if filler and kb2 >= start_iter:
                    n = -(-len(filler) // (NIT - kb2))
                    for _ in range(n):
                        filler.pop(0)()
                if prev is not None:
                    emit_ctx_pair(prev, kb2)
            if prev is not None:
                emit_norm(prev[0], prev[1], prev[3])
            for u in filler:
                u()
            pc_cur = ps_ctx.tile([128, SC], FP32, tag="pc")
            prev = (qc, h, P, pc_cur)

    nc.compile()
    return nc


def _get_nc():
    if "nc" not in _CACHE:
        _CACHE["nc"] = _build()
    return _CACHE["nc"]


def _tile_x(xb, chunk):
    # x (S, D) -> x^T tiled (S/chunk, 128, KT6, chunk), contiguous, bf16
    xt = np.asarray(xb, dtype=np.float32).T.astype(NP_CDT)
    return np.ascontiguousarray(
        xt.reshape(KT6, 128, S // chunk, chunk).transpose(2, 1, 0, 3)
    )


def _tile_w(w):
    # (D, G) -> (128, KT6, G) contiguous, bf16
    w = np.asarray(w, dtype=np.float32).astype(NP_CDT)
    return np.ascontiguousarray(w.reshape(KT6, 128, G).transpose(1, 0, 2))


def _in_maps(v, k, q, wq, bq, wk, bk, wv, bv, wo, bo):
    bf16 = lambda a: np.ascontiguousarray(
        np.asarray(a, dtype=np.float32).astype(NP_CDT)
    )
    qTb = [_tile_x(q[b], SC) for b in range(B)]
    kTb = [_tile_x(k[b], SC) for b in range(B)]
    vTb = [_tile_x(v[b], KB) for b in range(B)]
    maps = []
    for c in range(NCORES):
        b, g = divmod(c, NG)
        cols = slice(g * G, (g + 1) * G)
        # wk|wq side by side per t-tile (one wide, RR-friendly DMA).
        wkq_m = np.ascontiguousarray(np.concatenate(
            [_tile_w(np.asarray(w)[:, cols]) for w in (wk, wq)], axis=2
        ))
        # bias halves packed as columns: bk lo, bk hi, bq lo, bq hi.
        bp = np.zeros((128, 4), dtype=np.float32)
        bp[:, 0] = np.asarray(bk, dtype=np.float32)[cols][0:128]
        bp[0:64, 1] = np.asarray(bk, dtype=np.float32)[cols][128:G]
        bp[:, 2] = np.asarray(bq, dtype=np.float32)[cols][0:128]
        bp[0:64, 3] = np.asarray(bq, dtype=np.float32)[cols][128:G]
        maps.append({
            "qT": qTb[b],
            "kT": kTb[b],
            "vT": vTb[b],
            "wkq": wkq_m,
            "wv": _tile_w(np.asarray(wv)[:, cols]),
            "wo": bf16(wo[cols, :]),
            "bias": bp,
        })
    return maps


def kernel(v, k, q, wq, bq, wk, bk, wv, bv, wo, bo, _trace=False):
    nc = _get_nc()
    in_maps = _in_maps(v, k, q, wq, bq, wk, bk, wv, bv, wo, bo)
    res = bass_utils.run_bass_kernel_spmd(
        nc, in_maps, core_ids=list(range(NCORES)), trace=_trace
    )
    # softmax weights sum to 1, so the V bias shifts ctx by exactly bv;
    # its contribution to the output is the constant row bv @ wo + bo.
    corr = (np.asarray(bv, dtype=np.float64) @ np.asarray(wo, dtype=np.float64)
            + np.asarray(bo, dtype=np.float64)).astype(np.float32)
    out = np.empty((B, S, D), dtype=np.float32)
    for b in range(B):
        acc = res.results[4 * b]["yp"].astype(np.float32)
        for g in range(1, NG):
            acc = acc + res.results[4 * b + g]["yp"].astype(np.float32)
        out[b] = acc + corr[None, :]
    if _trace:
        kernel.last_result = res
    return out
